# revision 24
# baseline (speedup 1.0000x reference)
"""Multi-head self-attention (B=4, T=2048, C=1024, H=16) on 8 Trainium2 cores.

Sharding (head-split): core c handles batch b = c//2 and head-half
hh = c%2 (8 of the 16 heads), ALL 2048 queries and keys of its batch.
No K/V projection redundancy. The output projection contracts only this
core's 512 feature columns, so each core returns a PARTIAL [2048, 1024]
fp32 product; the host sums the two partials per batch and adds bo.

v2 schedule (trace-driven):
  - ScalarE exp of the 33.5M logits (256 x [128,1024] ACTIVATEs ~ 285us)
    is the floor; the kernel is organized to keep the ACT stream dense.
  - Startup: input DMA is priority-ordered (wv + X^T token-half 0 first)
    and spread over the sync/vector/gpsimd queues so the V projection
    starts at ~1.5us and the PE never idles long enough to re-arm the
    HAM throttle (the baseline lost ~20us to DMA holes + cold clock).
  - Flat slot loop with the score matmuls software-pipelined ONE slot
    ahead: S(k+1) is emitted before AV(k), so at a (hp,qp) pass boundary
    the next pass's first S pair no longer queues behind AV(kt15) (which
    hard-waits on the pass's last ACT) - this removes ~1.4us x 15 of
    ScalarE idle.
  - No DMA triggers on the scalar queue (each one stole ~0.7us from the
    ACT stream).
  - Tail: the output projection of qm 12..15 is two-phase - the hp0..2
    partial accumulations (24 MMs on 8 PSUM banks) run concurrently with
    the last normalize, only the 8 hp3 MMs wait for it.
"""
import sys

sys.path.insert(0, "/opt/trn_rl_repo")

from contextlib import ExitStack

import numpy as np

import concourse.bacc as bacc
import concourse.tile as tile
from concourse import library_config, mybir
from concourse.bass_utils import run_bass_kernel_spmd

F32 = mybir.dt.float32
BF16 = mybir.dt.bfloat16
AF = mybir.ActivationFunctionType

T, C, NH, D = 2048, 1024, 16, 64
HH = 8                  # heads per core
HF = HH * D             # 512 feature columns per core
P = 128
N_KC = C // P           # 8 contraction chunks
N_TT = T // P           # 16 token/key chunks
N_HP = HH // 2          # 4 head pairs per core
N_QP = 4                # query passes of 512
QW = T // N_QP          # 512 queries per pass
VW = D + 1              # per-head V width incl. ones column
SLOTS = N_HP * N_QP * N_TT  # 256

_CACHE = {}


def _build():
    nc = bacc.Bacc("TRN2", target_bir_lowering=False, debug=False)

    x = nc.declare_dram_parameter("x", [C, T], BF16, isOutput=False)  # X^T
    # wq/wk are host-pretransposed per head pair: [hp, p, kc, c] with
    # element [hp, p, kc*128+c] = W[kc*128+p, hp*128+c] so one DMA per hp
    # fills wq_t[:, :, hp*P:(hp+1)*P] directly.
    wq = nc.declare_dram_parameter("wq", [N_HP, P, N_KC, P], BF16, isOutput=False)
    wk = nc.declare_dram_parameter("wk", [N_HP, P, N_KC, P], BF16, isOutput=False)
    wv = nc.declare_dram_parameter("wv", [C, HF], BF16, isOutput=False)
    wo = nc.declare_dram_parameter("wo", [HF, C], BF16, isOutput=False)
    bq = nc.declare_dram_parameter("bq", [HF], F32, isOutput=False)
    bv_b = nc.declare_dram_parameter("bv_b", [P, HF], F32, isOutput=False)
    out = nc.declare_dram_parameter("out", [N_TT, P, C], F32, isOutput=True)

    with tile.TileContext(nc) as tc, ExitStack() as ctx:
        big = ctx.enter_context(tc.tile_pool(name="big", bufs=1))
        pt_pool = ctx.enter_context(tc.tile_pool(name="pt", bufs=8))
        rc_pool = ctx.enter_context(tc.tile_pool(name="rc", bufs=3))
        bc_pool = ctx.enter_context(tc.tile_pool(name="bc", bufs=3))
        s_ps = ctx.enter_context(tc.tile_pool(name="sps", bufs=2, space="PSUM"))
        o_ps = ctx.enter_context(tc.tile_pool(name="ops", bufs=2, space="PSUM"))
        pr_ps = ctx.enter_context(tc.tile_pool(name="prps", bufs=2, space="PSUM"))

        nc.gpsimd.load_library(library_config.attn)

        # ---- inputs to SBUF -------------------------------------------------
        # Priority-ordered DMA on three queues (scalar stays ACT-only):
        #   1. bv (gates the V bias adds), wv + X^T token-half 0 interleaved
        #      per kc so the V projection flows from ~1.5us,
        #   2. X^T half 1 (V tt 8..15), bq,
        #   3. wk/wq hp0 (K0/Q0 run at ~30us), then hp1..3, then wo.
        xt = big.tile([P, N_KC, T], BF16)          # X^T (c, t)
        wv_t = big.tile([P, N_KC, HF], BF16)
        wk_t = big.tile([P, N_KC, HF], BF16)
        wq_t = big.tile([P, N_KC, HF], BF16)
        bq_t = big.tile([P, N_HP], F32)
        bv_t = big.tile([P, HF], F32)
        wo_t = big.tile([P, N_HP, C], BF16)

        # The gpsimd (software-descriptor) DMA queue doesn't fire until
        # ~10us in, so everything the V projection / K0 / Q0 needs goes on
        # the two hardware queues (sync: X^T, scalar: wv then wk/wq hp0);
        # gpsimd only carries inputs first needed tens of us later.
        TH = T // 2
        for kc in range(N_KC):
            nc.scalar.dma_start(out=wv_t[:, kc, :], in_=wv[kc * P : (kc + 1) * P, :])
            nc.sync.dma_start(out=xt[:, kc, 0:TH], in_=x[kc * P : (kc + 1) * P, 0:TH])
        nc.scalar.dma_start(out=wk_t[:, :, 0:P], in_=wk[0])
        nc.sync.dma_start(out=bv_t[:, :], in_=bv_b[:, :])
        nc.sync.dma_start(out=wq_t[:, :, 0:P], in_=wq[0])
        for hp in range(N_HP):
            nc.scalar.dma_start(
                out=bq_t[:, hp : hp + 1], in_=bq[hp * P : (hp + 1) * P].unsqueeze(-1)
            )
        for kc in range(N_KC):
            eng = nc.sync if kc % 2 == 0 else nc.scalar
            eng.dma_start(out=xt[:, kc, TH:T], in_=x[kc * P : (kc + 1) * P, TH:T])
        for hp in range(1, N_HP):
            nc.gpsimd.dma_start(out=wk_t[:, :, hp * P : (hp + 1) * P], in_=wk[hp])
            nc.gpsimd.dma_start(out=wq_t[:, :, hp * P : (hp + 1) * P], in_=wq[hp])
        for hp in range(N_HP):
            nc.gpsimd.dma_start(out=wo_t[:, hp, :], in_=wo[hp * P : (hp + 1) * P, :])

        v_res = big.tile([P, N_TT, HH * VW], BF16)  # [v_h | 1] per head per chunk
        kt_res = big.tile([P, N_HP, T], BF16)       # K^T (f, t)
        qt = big.tile([P, N_HP, T], BF16)           # Q^T (f, q)
        attout = big.tile([P, N_HP, T], BF16)       # normalized O^T

        v_ones = v_res.rearrange("p t (h w) -> p t h w", w=VW)
        nc.vector.memset(v_ones[:, :, :, D : D + 1], 1.0)

        # ---- V = X @ Wv + bv, one token chunk at a time ---------------------
        # Only V(0) runs before the first ACT; V(1..15) are emitted
        # just-in-time inside pass-0 slots (V(tt) in slot tt-1, strictly
        # before AV(tt) so the PE FIFO dependency order stays acyclic).
        bv_v = bv_t.rearrange("p (h d) -> p h d", h=HH)

        def emit_v(tt):
            pv = pr_ps.tile([P, HF], F32, tag="pr")
            for kc in range(N_KC):
                nc.tensor.matmul(
                    pv[:, :],
                    xt[:, kc, tt * P : (tt + 1) * P],
                    wv_t[:, kc, :],
                    start=(kc == 0),
                    stop=(kc == N_KC - 1),
                )
            pv_v = pv.rearrange("p (h d) -> p h d", h=HH)
            nc.vector.tensor_add(v_ones[:, tt, :, 0:D], pv_v[:, :, :], bv_v[:, :, :])

        # ---- projection work generators (emitted inline with attention) ----
        def k_proj_steps(hp, pool=None, tag="pr", ths=tuple(range(N_QP))):
            """K^T(hp): per th-group, 8 accumulating MMs + a DVE cast."""
            for th in ths:
                pk = (pool or pr_ps).tile([P, QW], F32, tag=tag)
                for kc in range(N_KC):
                    yield lambda hp=hp, th=th, kc=kc, pk=pk: nc.tensor.matmul(
                        pk[:, :],
                        wk_t[:, kc, hp * P : (hp + 1) * P],
                        xt[:, kc, th * QW : (th + 1) * QW],
                        start=(kc == 0),
                        stop=(kc == N_KC - 1),
                    )
                yield lambda hp=hp, th=th, pk=pk: nc.vector.tensor_copy(
                    kt_res[:, hp, th * QW : (th + 1) * QW], pk[:, :]
                )

        def q_proj_steps(hp, pool=None, tag="pr", ths=tuple(range(N_QP))):
            for th in ths:
                pq = (pool or pr_ps).tile([P, QW], F32, tag=tag)
                for kc in range(N_KC):
                    yield lambda hp=hp, th=th, kc=kc, pq=pq: nc.tensor.matmul(
                        pq[:, :],
                        wq_t[:, kc, hp * P : (hp + 1) * P],
                        xt[:, kc, th * QW : (th + 1) * QW],
                        start=(kc == 0),
                        stop=(kc == N_KC - 1),
                    )
                yield lambda hp=hp, th=th, pq=pq: nc.vector.tensor_scalar_add(
                    qt[:, hp, th * QW : (th + 1) * QW], pq[:, :], bq_t[:, hp : hp + 1]
                )

        # no scalar-queue DMAs during the ACT span - each trigger is ~0.7us
        # of ScalarE time stolen from the exp stream
        odma = [nc.sync, nc.gpsimd]

        def out_proj_steps(qms):
            """Output projection for query chunks qms (contract all 4 hp)."""
            for qm in qms:
                for nh in range(2):
                    po = pr_ps.tile([P, QW], F32, tag="pr")
                    for hp in range(N_HP):
                        yield lambda qm=qm, nh=nh, hp=hp, po=po: nc.tensor.matmul(
                            po[:, :],
                            attout[:, hp, qm * P : (qm + 1) * P],
                            wo_t[:, hp, nh * QW : (nh + 1) * QW],
                            start=(hp == 0),
                            stop=(hp == N_HP - 1),
                        )

                    def _drain(qm=qm, nh=nh, po=po):
                        os_ = bc_pool.tile([P, QW], F32, tag="os")
                        nc.vector.tensor_copy(os_[:, :], po[:, :])
                        odma[(2 * qm + nh) % 2].dma_start(
                            out=out[qm, :, nh * QW : (nh + 1) * QW], in_=os_[:, :]
                        )

                    yield _drain

        def chain(*gens):
            for g in gens:
                yield from g

        # upfront: V(0), K^T(0, th0), Q^T(0, th0) only - just enough for the
        # first ACT at ~17us instead of ~63us. K0/Q0 use the s pool (idle
        # until attention starts).

        def _s_pool_qw():
            class p:
                @staticmethod
                def tile(shape, dt_, tag=None):
                    t = s_ps.tile([P, 2 * QW], dt_, tag="s")
                    return t[:, 0 : shape[1]]
            return p

        emit_v(0)
        for step in chain(
            k_proj_steps(0, pool=_s_pool_qw(), ths=(0,)),
            q_proj_steps(0, pool=_s_pool_qw(), ths=(0,)),
        ):
            step()

        # deferred startup work, emitted at fixed pass-0 slots. V(tt) must be
        # emitted before AV(tt) runs; K0(th) before S(4*th) is emitted;
        # Q0(th1) before S(16) is emitted. Pass 0 runs PE-bound (~2.5us per
        # slot), but its 16 ACTs overlap work that previously ran with
        # ScalarE fully idle.
        pass0_extra = {
            1: list(k_proj_steps(0, ths=(1,))),
            3: list(k_proj_steps(0, ths=(2,))),
            5: list(k_proj_steps(0, ths=(3,))),
            8: list(q_proj_steps(0, ths=(1,))),
        }

        # side work emitted during attention slots: K/Q of head pair hp+1
        # front-loaded into hp's passes; out-proj of qp-1's chunks during
        # hp3's passes qp>=1 (needs ALL head pairs' attout).
        side = {}
        g0 = chain(
            q_proj_steps(0, ths=(2,)), q_proj_steps(0, ths=(3,)),
            k_proj_steps(1), q_proj_steps(1),
        )
        side[(0, 0)] = (iter(()), 0.0)
        for qp in range(1, N_QP):
            side[(0, qp)] = (g0, 90 / 48)
        for hp in range(1, 3):
            g = chain(k_proj_steps(hp + 1), q_proj_steps(hp + 1))
            for qp in range(N_QP):
                side[(hp, qp)] = (g, 72 / 64)
        side[(3, 0)] = (iter(()), 0.0)
        for qp in range(1, N_QP):
            # 4 qm x (8 MMs + 2 drains) = 40 steps over 16 slots
            side[(3, qp)] = (out_proj_steps(range(4 * (qp - 1), 4 * qp)), 40 / 16)

        # ---- attention: flat slot loop, S one slot ahead --------------------
        def emit_s(idx):
            hp, qp, kt = idx // 64, (idx // 16) % 4, idx % 16
            q0 = qp * QW
            s = s_ps.tile([P, 2 * QW], F32, tag="s")
            nc.tensor.matmul(
                s[:, 0:QW],
                kt_res[0:64, hp, kt * P : (kt + 1) * P],
                qt[0:64, hp, q0 : q0 + QW],
                start=True,
                stop=True,
                tile_position=(0, 0),
            )
            nc.tensor.matmul(
                s[:, QW : 2 * QW],
                kt_res[64:128, hp, kt * P : (kt + 1) * P],
                qt[64:128, hp, q0 : q0 + QW],
                start=True,
                stop=True,
                tile_position=(64, 0),
            )
            return s

        def normalize(hp, qp, oA, oB):
            # attout[d, q] = O[d, q] / O[64, q]. Copy O and den out of PSUM
            # FIRST so the O banks free early; the recip -> gpsimd-broadcast
            # -> mul chain then runs off the PSUM critical path.
            q0 = qp * QW
            chains = []
            for row0, o_t in ((0, oA), (64, oB)):
                o_sb = bc_pool.tile([64, QW], F32, tag="osb")
                nc.vector.tensor_copy(o_sb[:, :], o_t[0:64, :])
                den_t = rc_pool.tile([1, QW], F32, tag="den")
                nc.vector.tensor_copy(den_t[:, :], o_t[64:VW, :])
                chains.append((row0, o_sb, den_t))
            for row0, o_sb, den_t in chains:
                rc_t = rc_pool.tile([1, QW], F32, tag="rc")
                nc.vector.reciprocal_approx_fast(out=rc_t[:, :], in_=den_t[:, :])
                bc_t = bc_pool.tile([64, QW], F32, tag="bc")
                nc.gpsimd.partition_broadcast(bc_t[:, :], rc_t[:, :])
                nc.vector.tensor_mul(
                    attout[row0 : row0 + 64, hp, q0 : q0 + QW],
                    o_sb[:, :],
                    bc_t[:, :],
                )

        pending_s = emit_s(0)
        gen, side_per_iter, quota = iter(()), 0.0, 0.0
        oA = oB = None
        for k in range(SLOTS):
            hp, qp, kt = k // 64, (k // 16) % 4, k % 16
            if kt == 0:
                oA = o_ps.tile([VW, QW], F32, tag="o")
                oB = o_ps.tile([VW, QW], F32, tag="o")
                gen, side_per_iter = side[(hp, qp)]
                quota = 0.0
            hA, hB = 2 * hp, 2 * hp + 1
            s = pending_s
            p_t = pt_pool.tile([P, 2 * QW], BF16, tag="pt")
            nc.scalar.activation(p_t[:, :], s[:, :], AF.Exp, scale=0.125)
            # next slot's scores FIRST: everything they need is ready, so
            # they run at slot start and the next ACT is never starved -
            # side work and the ACT-gated AV queue behind them
            if k + 1 < SLOTS:
                pending_s = emit_s(k + 1)
            if k < 15:
                # pass-0 JIT startup: V(k+1) before AV(k+1) exists, plus
                # the scheduled K0/Q0 th-groups
                for step in pass0_extra.get(k, ()):
                    step()
                emit_v(k + 1)
            quota += side_per_iter
            while quota >= 1.0:
                step = next(gen, None)
                if step is None:
                    quota = 0.0
                    break
                step()
                quota -= 1.0
            nc.tensor.matmul(
                oA[:, :],
                v_res[:, kt, hA * VW : (hA + 1) * VW],
                p_t[:, 0:QW],
                start=(kt == 0),
                stop=(kt == N_TT - 1),
            )
            nc.tensor.matmul(
                oB[:, :],
                v_res[:, kt, hB * VW : (hB + 1) * VW],
                p_t[:, QW : 2 * QW],
                start=(kt == 0),
                stop=(kt == N_TT - 1),
            )
            if kt == N_TT - 1:
                normalize(hp, qp, oA, oB)
                # drain leftover side work (shared gens span all 4 passes)
                if qp == N_QP - 1 or hp == 3:
                    for step in gen:
                        step()

        # ---- output projection tail (qm 12..15) -----------------------------
        # Two-phase: hp0..2 partial accumulations for all 8 (qm, nh) chains
        # run on 8 PSUM banks (s + pr + o pools, all free by now) while the
        # last normalize finishes - this also keeps the PE warm through the
        # normalize chain (an idle PE re-throttles to half clock in ~3.4us);
        # only the 8 hp3 MMs wait for it.
        tail_chains = [(qm, nh) for qm in range(12, N_TT) for nh in range(2)]
        sfA = s_ps.tile([P, 2 * QW], F32, tag="s")
        sfB = s_ps.tile([P, 2 * QW], F32, tag="s")
        tl_pr0 = pr_ps.tile([P, QW], F32, tag="pr")
        tl_pr1 = pr_ps.tile([P, QW], F32, tag="pr")
        tl_o0 = o_ps.tile([P, QW], F32, tag="o")
        tl_o1 = o_ps.tile([P, QW], F32, tag="o")
        po_slots = [
            sfA[:, 0:QW], sfA[:, QW : 2 * QW],
            sfB[:, 0:QW], sfB[:, QW : 2 * QW],
            tl_pr0, tl_pr1, tl_o0, tl_o1,
        ]
        for hp in range(3):
            for c, (qm, nh) in enumerate(tail_chains):
                nc.tensor.matmul(
                    po_slots[c][:, :],
                    attout[:, hp, qm * P : (qm + 1) * P],
                    wo_t[:, hp, nh * QW : (nh + 1) * QW],
                    start=(hp == 0),
                    stop=False,
                )
        for c, (qm, nh) in enumerate(tail_chains):
            nc.tensor.matmul(
                po_slots[c][:, :],
                attout[:, 3, qm * P : (qm + 1) * P],
                wo_t[:, 3, nh * QW : (nh + 1) * QW],
                start=False,
                stop=True,
            )
            os_ = bc_pool.tile([P, QW], F32, tag="os")
            # ScalarE is idle after the last ACT - split the drain copies
            # between it and the DVE so they pipeline behind the hp3 MMs
            if c % 2 == 0:
                nc.vector.tensor_copy(os_[:, :], po_slots[c][:, :])
            else:
                nc.scalar.copy(os_[:, :], po_slots[c][:, :])
            odma[c % 2].dma_start(
                out=out[qm, :, nh * QW : (nh + 1) * QW], in_=os_[:, :]
            )

    nc.finalize()
    return nc


def _get_program():
    if "nc" not in _CACHE:
        _CACHE["nc"] = _build()
    return _CACHE["nc"]


def _bf16(a):
    import ml_dtypes

    return np.asarray(a, np.float32).astype(ml_dtypes.bfloat16)


def _w4(w_h):
    """[C, HF] slice -> [N_HP, P, N_KC, P] with [hp, p, kc, c] = W[kc*P+p, hp*P+c]."""
    return np.ascontiguousarray(
        w_h.reshape(N_KC, P, N_HP, P).transpose(2, 1, 0, 3)
    )


def kernel(x, Wq, bq, Wk, bk, Wv, bv, Wo, bo, _trace=False, _trace_kwargs=None):
    x = np.asarray(x, np.float32)
    bq, bv, bo = (np.asarray(b, np.float32) for b in (bq, bv, bo))
    # bk unused: a key-side bias adds a per-query constant to every logit of a
    # softmax row, which cancels exactly in the softmax.

    x_b = [np.ascontiguousarray(_bf16(x[b]).T) for b in range(4)]
    wq_h = [_w4(_bf16(Wq[:, h * HF : (h + 1) * HF])) for h in range(2)]
    wk_h = [_w4(_bf16(Wk[:, h * HF : (h + 1) * HF])) for h in range(2)]
    wv_h = [_bf16(Wv[:, h * HF : (h + 1) * HF]) for h in range(2)]
    wo_h = [np.ascontiguousarray(_bf16(Wo[h * HF : (h + 1) * HF, :])) for h in range(2)]
    bq_h = [np.ascontiguousarray(bq[h * HF : (h + 1) * HF]) for h in range(2)]
    bv_h = [
        np.ascontiguousarray(np.broadcast_to(bv[h * HF : (h + 1) * HF], (P, HF)))
        for h in range(2)
    ]

    nc = _get_program()
    in_maps = []
    for c in range(8):
        b, hh = divmod(c, 2)
        in_maps.append(
            {
                "x": x_b[b],
                "wq": wq_h[hh], "wk": wk_h[hh], "wv": wv_h[hh],
                "wo": wo_h[hh], "bq": bq_h[hh], "bv_b": bv_h[hh],
            }
        )

    kw = {}
    if _trace:
        kw = dict(trace=True, **(_trace_kwargs or {}))
    res = run_bass_kernel_spmd(nc, in_maps, list(range(8)), **kw)
    _CACHE["last_result"] = res

    outp = np.empty((4, T, C), np.float32)
    for b in range(4):
        p0 = res.results[2 * b]["out"].reshape(T, C)
        p1 = res.results[2 * b + 1]["out"].reshape(T, C)
        outp[b] = p0 + p1
    outp += bo.astype(np.float32)
    return outp
